# revision 1
# baseline (speedup 1.0000x reference)
"""Trainium2 Bass kernel for nn_DeterministicEncoder (MLP encoder + Laplace-kernel
attention).

  ctx = [x_context, y_context]            # [M, 2]
  h   = relu(ctx @ W1 + b1); h = relu(h @ W2 + b2); v = h @ W3 (+ b3=0)
  out[n] = sum_m exp(-|k_m - q_n|) * v[m],  k = x_context, q = x_target

Factorized attention (no per-element exp):
  exp(-|k-q|) = [q>=k] * e^k * e^-q  +  [q<k] * e^-k * e^q
so with av = e^k * v and bv = e^-k * v (folded into the MLP's h2 via a
column scale), one 0/1 mask matrix per m-tile drives a single packed matmul
  lhsT = [av | bv]  (128 cols) , rhs = mask [128, n]  (+ a ones column
accumulating S = sum_m bv), and the host combines
  out[n] = e^-q * P1[n] + e^q * (S - P2[n]).

Sharding: x_target split across 8 cores (1024 rows each); everything else
replicated. Host prep is pure relayout/dtype-cast of inputs.
"""

import numpy as np
import ml_dtypes

import concourse.bass as bass
import concourse.tile as tile
from concourse import mybir
from concourse.bass_utils import run_bass_kernel_spmd

N_CORES = 8
M = 8192
N = 8192
N_SH = N // N_CORES
H = 16
OUT = 64
NG = 8              # m-groups stacked across partitions in the MLP
MJ = M // NG        # 1024
MT = M // 128       # 64 m-tiles
FB = 4              # m-tiles per mask batch
FAST = True

F32 = mybir.dt.float32
BF16 = mybir.dt.bfloat16

# blob column offsets (f32 lanes)
QB0 = 0                      # q broadcast [N_SH]
KC0 = QB0 + N_SH             # k-columns f32 [MT] (epilogue use)
B10 = KC0 + MT               # b1s, b2s
XR0 = B10 + 2                # xrep [MJ]: xc[(p//16)*MJ + j]
QB16 = XR0 + MJ              # q broadcast as bf16 pairs [N_SH/2]
KC16 = QB16 + N_SH // 2      # k-columns as bf16 pairs [MT/2]
CTX0 = KC16 + MT // 2        # rows 0-15: stacked context
W10 = CTX0 + MJ              # rows 0-15: W1 blockdiag
W20 = W10 + 128              # W2 blockdiag
W30 = W20 + 128              # W3 stacked per group (bf16 pairs if fast)


def _build(fast: bool, legalize: bool = True) -> bass.Bass:
    wdt = BF16 if fast else F32
    nc = bass.Bass()
    W3C = NG * OUT // (2 if fast else 1)
    BLOB_W = W30 + W3C
    blob_d = nc.dram_tensor("blob", [128, BLOB_W], F32, kind="ExternalInput")
    out_d = nc.dram_tensor("out", [128, N_SH], F32, kind="ExternalOutput")

    with tile.TileContext(nc) as tc:
        with (
            tc.tile_pool(name="const", bufs=1) as const,
            tc.tile_pool(name="vbuf", bufs=1) as vbuf,
            tc.tile_pool(name="mbuf", bufs=2) as mbuf,
            tc.tile_pool(name="mlpps", bufs=1, space="PSUM") as mlpps,
            tc.tile_pool(name="vps", bufs=2, space="PSUM") as vps,
            tc.tile_pool(name="ops", bufs=1, space="PSUM") as ops,
        ):
            blob = const.tile([128, BLOB_W], F32)
            nc.sync.dma_start(out=blob[:], in_=blob_d[:, :])
            qb = blob[:, QB0:QB0 + N_SH]
            b1 = blob[:, B10:B10 + 1]
            b2 = blob[:, B10 + 1:B10 + 2]
            xrep = blob[:, XR0:XR0 + MJ]
            ctxs = blob[0:2 * NG, CTX0:CTX0 + MJ]
            w1 = blob[0:2 * NG, W10:W10 + 128]
            w2 = blob[:, W20:W20 + 128]
            w3raw = blob[:, W30:W30 + W3C]
            w3 = w3raw.bitcast(wdt) if fast else w3raw
            # comparison ops require an fp32 scalar operand
            kmask = blob[:, KC0:KC0 + MT]
            qmask = (blob[:, QB16:QB16 + N_SH // 2].bitcast(BF16)
                     if fast else qb)
            # early DVE op consuming the blob DMA (advances DVE's DMA tick)
            qk_seen = const.tile([128, 1], F32)
            nc.vector.tensor_copy(qk_seen[:], blob[:, 0:1])

            # ---- MLP layer 1/2 (8 m-groups stacked; relu+bias fused on DVE)
            ps1 = mlpps.tile([128, MJ], F32, tag="ps")
            for c in range(MJ // 512):
                nc.tensor.matmul(ps1[:, bass.ts(c, 512)], w1[:],
                                 ctxs[:, bass.ts(c, 512)], start=True, stop=True)
            h1 = const.tile([128, MJ], F32)
            nc.vector.tensor_scalar(out=h1[:], in0=ps1[:], scalar1=b1[:],
                                    scalar2=0.0, op0=mybir.AluOpType.add,
                                    op1=mybir.AluOpType.max)
            ps2 = mlpps.tile([128, MJ], F32, tag="ps")
            for c in range(MJ // 512):
                nc.tensor.matmul(ps2[:, bass.ts(c, 512)], w2[:],
                                 h1[:, bass.ts(c, 512)], start=True, stop=True)
            h2 = const.tile([128, MJ], F32)
            nc.vector.tensor_scalar(out=h2[:], in0=ps2[:], scalar1=b2[:],
                                    scalar2=0.0, op0=mybir.AluOpType.add,
                                    op1=mybir.AluOpType.max)

            # ---- column scales e^{+-k} folded into h2 (A on ACT, mult on DVE)
            A1 = const.tile([128, MJ], F32)
            nc.scalar.activation(A1[:], xrep, mybir.ActivationFunctionType.Exp,
                                 scale=1.0)
            A2 = const.tile([128, MJ], F32)
            nc.scalar.activation(A2[:], xrep, mybir.ActivationFunctionType.Exp,
                                 scale=-1.0)
            h2a = const.tile([128, MJ], wdt)
            nc.vector.tensor_mul(h2a[:], h2[:], A1[:])
            h2b = const.tile([128, MJ], wdt)
            nc.vector.tensor_mul(h2b[:], h2[:], A2[:])

            # ---- row factors e^{-q} (rows 0-63) / e^{+q} (rows 64-127)
            bq2 = const.tile([128, N_SH], F32)
            nc.scalar.activation(bq2[0:OUT, :], qb[0:OUT, :],
                                 mybir.ActivationFunctionType.Exp, scale=-1.0)
            nc.scalar.activation(bq2[OUT:128, :], qb[OUT:128, :],
                                 mybir.ActivationFunctionType.Exp, scale=1.0)
            # DVE pre-consume of ACT prologue tick
            bq_seen = const.tile([128, 1], F32)
            nc.vector.tensor_copy(bq_seen[:], bq2[:, 0:1])

            # ---- v-layers: packed lhsT per m-tile t: [av(64) | bv(64)]
            # m = g*MJ + jt*128 + p ; t = g*8 + jt
            v_sb = vbuf.tile([128, MT * 128], wdt)
            v_vw = v_sb[:].rearrange("p (t c) -> p t c", c=128)
            for half, h2x in ((0, h2a), (1, h2b)):
                for jt in range(8):
                    pv = vps.tile([128, NG * OUT], F32)
                    # w3 is block-diagonal per group, so ONE N=512 matmul
                    # yields all 8 groups' v-tiles side by side.
                    nc.tensor.matmul(pv[:], h2x[:, bass.ts(jt, 128)], w3[:],
                                     start=True, stop=True)
                    dst = v_vw[:, jt::8, half * OUT:(half + 1) * OUT]
                    nc.scalar.copy(dst, pv[:].rearrange("p (g c) -> p g c", c=OUT))

            # PE pre-consume of v_sb (absorbs the ACT wait so the first real
            # attention matmul carries only the DVE mask wait)
            scr = mlpps.tile([1, 1], F32, tag="scr")
            nc.tensor.matmul(scr[:], v_sb[0:1, 0:1], v_sb[0:1, 0:1],
                             start=True, stop=True)

            # ---- attention: one mask stream per m-tile; psum rows 0-63 = P1
            # (sum mask*av), rows 64-127 = P2 (sum mask*bv); col 1024 via the
            # ones column accumulates [S_av | S_bv]
            MW = N_SH + 1
            po = ops.tile([128, MW], F32)
            for b in range(MT // FB):
                m = mbuf.tile([128, FB * MW], wdt)
                mv = m[:].rearrange("p (i c) -> p i c", c=MW)
                nc.vector.memset(mv[:, :, N_SH:N_SH + 1], 1.0)
                for i in range(FB):
                    t = b * FB + i
                    nc.vector.tensor_scalar(
                        out=m[:, i * MW:i * MW + N_SH], in0=qmask,
                        scalar1=kmask[:, t:t + 1], scalar2=None,
                        op0=mybir.AluOpType.is_ge)
                for i in range(FB):
                    t = b * FB + i
                    for u in range(N_SH // 512):
                        nc.tensor.matmul(
                            po[:, bass.ts(u, 512)], v_vw[:, t:t + 1, :],
                            m[:, i * MW + u * 512:i * MW + (u + 1) * 512],
                            start=(t == 0), stop=(t == MT - 1))
                    nc.tensor.matmul(
                        po[:, N_SH:N_SH + 1], v_vw[:, t:t + 1, :],
                        m[:, i * MW + N_SH:i * MW + MW],
                        start=(t == 0), stop=(t == MT - 1))

            # ---- epilogue: top = e^-q * P1 ; bot = (P2 - S_bv) * e^q
            # host computes out = (top - bot).T
            scol = const.tile([128, 1], F32)
            nc.vector.tensor_copy(scol[OUT:128, :], po[OUT:128, N_SH:N_SH + 1])
            o_sb = vbuf.tile([128, N_SH], F32)
            nc.vector.tensor_mul(o_sb[0:OUT, :], po[0:OUT, 0:N_SH], bq2[0:OUT, :])
            nc.vector.scalar_tensor_tensor(
                out=o_sb[OUT:128, :], in0=po[OUT:128, 0:N_SH],
                scalar=scol[OUT:128, :], in1=bq2[OUT:128, :],
                op0=mybir.AluOpType.subtract, op1=mybir.AluOpType.mult)
            nc.sync.dma_start(out=out_d[:, :], in_=o_sb[:])

    if legalize:
        _fix_tsp_waits(nc)
    return nc


def _fix_tsp_waits(nc: bass.Bass) -> None:
    """Walrus accepts at most ONE sync-wait per compute instruction (and few
    on the tail drain). Same-engine self-waits are redundant — every engine
    completes its queue strictly in order — so drop them; the tail drain
    keeps only the output-DMA wait (the dag funnels through it)."""
    budget = {"InstTensorScalarPtr": 1, "InstMatmult": 1, "InstTensorCopy": 1,
              "InstMemset": 1, "InstActivation": 1}
    eng_prefix = {"DVE": "DVE_", "Activation": "Activation_", "PE": "PE_",
                  "SP": "SP_", "Pool": "Pool_"}
    blocks = nc.m.functions[0].blocks
    out_dma_sems: set[str] = set()
    for b in blocks:
        for inst in b.instructions:
            if type(inst).__name__ == "InstDMACopy" and inst.sync_info:
                out_dma_sems = {u.ant_name for u in inst.sync_info.on_update
                                if u.ant_name}
    for b in blocks:
        for inst in b.instructions:
            tname = type(inst).__name__
            si = inst.sync_info
            if si is None:
                continue
            if tname == "InstDrain" and len(si.on_wait) > 1:
                kept = [w for w in si.on_wait if w.ant_name in out_dma_sems]
                if len(kept) != 1:
                    raise RuntimeError(f"tail drain {inst.name}: waits "
                                       f"{[(w.ant_name, w.wait_value) for w in si.on_wait]}")
                si.on_wait = kept
                inst.sync_info = si
                continue
            lim = budget.get(tname)
            if lim is None or len(si.on_wait) <= lim:
                continue
            eng = str(inst.engine).split(".")[-1]
            pfx = eng_prefix.get(eng, "\x00")
            kept = [w for w in si.on_wait
                    if not (w.ant_name or "").startswith(pfx)]
            if len(kept) > lim:
                raise RuntimeError(
                    f"{inst.name} ({tname}, {eng}): "
                    f"{[(w.ant_name, w.wait_value) for w in si.on_wait]}")
            si.on_wait = kept
            inst.sync_info = si


def _prep_maps(inputs: dict, fast: bool) -> list[dict]:
    xc = np.ascontiguousarray(inputs["x_context"], dtype=np.float32).reshape(M)
    yc = np.ascontiguousarray(inputs["y_context"], dtype=np.float32).reshape(M)
    xt = np.ascontiguousarray(inputs["x_target"], dtype=np.float32).reshape(N)
    W1 = np.asarray(inputs["W1"], dtype=np.float32)
    b1 = np.asarray(inputs["b1"], dtype=np.float32)
    W2 = np.asarray(inputs["W2"], dtype=np.float32)
    b2 = np.asarray(inputs["b2"], dtype=np.float32)
    W3 = np.asarray(inputs["W3"], dtype=np.float32)

    kct = np.ascontiguousarray(xc.reshape(MT, 128).T)          # [128, MT]
    xrep = np.repeat(xc.reshape(NG, MJ), H, axis=0)  # [128, MJ]
    ctxs = np.empty((2 * NG, MJ), dtype=np.float32)
    ctxs[0::2] = xc.reshape(NG, MJ)
    ctxs[1::2] = yc.reshape(NG, MJ)
    w1bd = np.zeros((2 * NG, 128), dtype=np.float32)
    w2bd = np.zeros((128, 128), dtype=np.float32)
    w3stk = np.zeros((128, NG * OUT), dtype=np.float32)
    for g in range(NG):
        w1bd[2 * g:2 * g + 2, H * g:H * (g + 1)] = W1
        w2bd[H * g:H * (g + 1), H * g:H * (g + 1)] = W2
        w3stk[H * g:H * (g + 1), OUT * g:OUT * (g + 1)] = W3
    b1s = np.tile(b1, NG).astype(np.float32)
    b2s = np.tile(b2, NG).astype(np.float32)

    if fast:
        w3f32 = np.ascontiguousarray(w3stk.astype(ml_dtypes.bfloat16)).view(np.float32)
        kct16 = np.ascontiguousarray(kct.astype(ml_dtypes.bfloat16)).view(np.float32)
    else:
        w3f32 = w3stk
        kct16 = np.zeros((128, MT // 2), dtype=np.float32)
    W3C = w3f32.shape[1]

    maps = []
    for c in range(N_CORES):
        q = xt[c * N_SH:(c + 1) * N_SH]
        blob = np.zeros((128, W30 + W3C), dtype=np.float32)
        blob[:, QB0:QB0 + N_SH] = q[None, :]
        blob[:, KC0:KC0 + MT] = kct
        blob[:, B10] = b1s
        blob[:, B10 + 1] = b2s
        blob[:, XR0:XR0 + MJ] = xrep
        if fast:
            q16 = np.ascontiguousarray(
                np.broadcast_to(q[None, :], (128, N_SH)).astype(ml_dtypes.bfloat16)
            ).view(np.float32)
            blob[:, QB16:QB16 + N_SH // 2] = q16
            blob[:, KC16:KC16 + MT // 2] = kct16
        blob[0:2 * NG, CTX0:CTX0 + MJ] = ctxs
        blob[0:2 * NG, W10:W10 + 128] = w1bd
        blob[:, W20:W20 + 128] = w2bd
        blob[:, W30:W30 + W3C] = w3f32
        maps.append({"blob": blob})
    return maps


def _unshard(results: list[dict], b3: np.ndarray) -> np.ndarray:
    if np.any(np.asarray(b3)):
        raise RuntimeError("nonzero b3 unsupported by the packed kernel")
    out = np.empty((N, OUT), dtype=np.float32)
    for c in range(N_CORES):
        o = results[c]["out"]                       # [128, N_SH]
        out[c * N_SH:(c + 1) * N_SH] = (o[:OUT] - o[OUT:]).T
    return out


def run(inputs: dict, fast: bool = FAST, **spmd_kwargs):
    nc = _build(fast)
    in_maps = _prep_maps(inputs, fast)
    res = run_bass_kernel_spmd(nc, in_maps, list(range(N_CORES)), **spmd_kwargs)
    return _unshard(res.results, inputs["b3"]), res


def kernel(**inputs) -> np.ndarray:
    out, _ = run(inputs, FAST)
    return out

